# revision 1
# baseline (speedup 1.0000x reference)
"""Causal multi-head attention (B=2, T=2048, D=1024, NH=16, HD=64) on 8 trn2
NeuronCores.

Sharding: data-parallel over batch (2) x tensor-parallel over head groups (4),
Megatron-style. Core c handles batch c//4, heads 4*(c%4)..4*(c%4)+3: it
computes qkv with the column slice of w_qkv for its heads, full causal
attention for those heads, and the partial output projection with the matching
row slice of w_proj. The host sums the 4 partial projections per batch.

On-device layout is feature-on-partition ("transposed") throughout:
  qk^T [512, T], S^T [k, q] blocks, attention output O^T, final out^T.
The host transposes x on the way in and the partial outputs on the way out.

Matmuls run as float32r (full PE rate, ~tf32-ish rounding, rel err ~1.5e-4 per
matmul). Softmax skips max-subtraction (scores are O(1) by construction), and
the causal mask is applied by zeroing exp(S) on diagonal blocks via gpsimd
affine_select (exp(-1e9) == 0 in the reference, so results match). The softmax
denominator comes free from a ones column appended to V (PV matmul row 64 =
sum_k P). S^T matmuls for the two heads of a pair are row-packed into the same
PE windows via tile_position (contraction is only 64).
"""

import sys

if "/opt/trn_rl_repo" not in sys.path:
    sys.path.insert(0, "/opt/trn_rl_repo")

import numpy as np
import concourse.mybir as mybir
from concourse import bacc
from concourse.tile import TileContext
from concourse import bass_utils

B, T, D = 2, 2048, 1024
NH, HD = 16, 64
HL = 4  # heads per core
N_CORES = 8

KT = D // 128  # 8 contraction tiles over model dim
TCH = T // 512  # 4 q-chunks of 512
TT = T // 128  # 16 t-blocks of 128
KG = 2  # S^T k-blocks per psum group

F32R = mybir.dt.float32r
F32 = mybir.dt.float32


def build_nc():
    nc = bacc.Bacc()
    xT = nc.dram_tensor("xT", [D, T], F32R, kind="ExternalInput")
    wqk = nc.dram_tensor("wqk", [D, 512], F32R, kind="ExternalInput")
    wv = nc.dram_tensor("wv", [D, 256], F32R, kind="ExternalInput")
    wp = nc.dram_tensor("wp", [256, D], F32R, kind="ExternalInput")
    onesc = nc.dram_tensor("onesc", [128, HL], F32R, kind="ExternalInput")
    outT = nc.dram_tensor("outT", [D, T], F32, kind="ExternalOutput")

    with TileContext(nc) as tc:
        with (
            tc.tile_pool(name="persist", bufs=1) as pers,
            tc.tile_pool(name="small", bufs=1) as spool,
        ):
            qkT_sb = [
                pers.tile([128, T], F32R, tag=f"qkT{mt}", name=f"qkT{mt}")
                for mt in range(4)
            ]
            # V1[tt]: [128 t, 4 heads, 65] -- col 64 is the ones column
            V1_sb = [
                pers.tile([128, HL, 65], F32R, tag=f"V1_{tt}", name=f"V1_{tt}")
                for tt in range(TT)
            ]
            AT_sb = [
                pers.tile([128, T], F32R, tag=f"AT{p}", name=f"AT{p}")
                for p in range(2)
            ]
            wp_sb = [
                pers.tile([128, D], F32R, tag=f"wp{ft}", name=f"twp{ft}")
                for ft in range(2)
            ]

            # ---- phase A: qk^T = wqk.T @ x (m-tile order: pair-0 first),
            # ---- phase B: V natural = x @ wv --------------------------------
            with (
                tc.tile_pool(name="qkv_in", bufs=KT) as qin,
                tc.tile_pool(name="psA", bufs=3, space="PSUM") as psa_pool,
                tc.tile_pool(name="psB", bufs=2, space="PSUM") as psb_pool,
            ):
                wqk_sb, wv_sb, xT_sb = [], [], []
                for kt in range(KT):
                    twqk = qin.tile([128, 512], F32R, tag="wqk")
                    nc.sync.dma_start(
                        out=twqk, in_=wqk[kt * 128 : (kt + 1) * 128, :]
                    )
                    wqk_sb.append(twqk)
                    tx = qin.tile([128, T], F32R, tag="xT")
                    for hh in range(2):
                        dma_eng = [nc.sync, nc.scalar][(kt + hh) % 2]
                        dma_eng.dma_start(
                            out=tx[:, hh * 1024 : (hh + 1) * 1024],
                            in_=xT[
                                kt * 128 : (kt + 1) * 128,
                                hh * 1024 : (hh + 1) * 1024,
                            ],
                        )
                    xT_sb.append(tx)
                for kt in range(KT):
                    twv = qin.tile([128, 256], F32R, tag="wv")
                    nc.sync.dma_start(out=twv, in_=wv[kt * 128 : (kt + 1) * 128, :])
                    wv_sb.append(twv)
                for ft in range(2):
                    nc.sync.dma_start(
                        out=wp_sb[ft], in_=wp[ft * 128 : (ft + 1) * 128, :]
                    )
                for tt in range(TT):
                    nc.sync.dma_start(
                        out=V1_sb[tt][:, :, 64:65], in_=onesc[:, :, None]
                    )

                for i, mt in enumerate([0, 2, 1, 3]):
                    for half in range(2):
                        ps = psa_pool.tile(
                            [128, 1024], F32, tag="qk", name=f"qkps{mt}_{half}"
                        )
                        for kt in range(KT):
                            for t2 in range(2):
                                nc.tensor.matmul(
                                    ps[:, t2 * 512 : (t2 + 1) * 512],
                                    wqk_sb[kt][:, mt * 128 : (mt + 1) * 128],
                                    xT_sb[kt][
                                        :,
                                        half * 1024
                                        + t2 * 512 : half * 1024
                                        + (t2 + 1) * 512,
                                    ],
                                    start=(kt == 0),
                                    stop=(kt == KT - 1),
                                )
                        if (2 * i + half) % 2 == 0:
                            nc.vector.tensor_copy(
                                qkT_sb[mt][:, half * 1024 : (half + 1) * 1024], ps
                            )
                        else:
                            nc.scalar.copy(
                                qkT_sb[mt][:, half * 1024 : (half + 1) * 1024], ps
                            )

                for tt in range(TT):
                    psv = psb_pool.tile([128, 256], F32, tag="v", name=f"vps{tt}")
                    for kt in range(KT):
                        nc.tensor.matmul(
                            psv[:, :],
                            xT_sb[kt][:, tt * 128 : (tt + 1) * 128],
                            wv_sb[kt][:, :],
                            start=(kt == 0),
                            stop=(kt == KT - 1),
                        )
                    if tt % 2 == 0:
                        nc.vector.tensor_copy(V1_sb[tt][:, :, 0:64], psv)
                    else:
                        nc.scalar.copy(V1_sb[tt][:, :, 0:64], psv)


            # ---- attention as one global software pipeline over (qc, kb)
            # steps: S/exp lead, affine+PV lag by DEPTH, staging/normalize
            # emitted inline at the lagged position, projection spread in
            # single-tile pieces between S steps -----------------------------
            with (
                tc.tile_pool(name="ptile", bufs=7) as ppool,
                tc.tile_pool(name="stage", bufs=1) as stg,
                tc.tile_pool(name="psS", bufs=2, space="PSUM") as pss_pool,
                tc.tile_pool(name="psO", bufs=4, space="PSUM") as pso_pool,
            ):
                QC_ORDER = [0, 3, 2, 1]
                steps = [(qc, kb) for qc in QC_ORDER for kb in range(4 * qc + 4)]
                DEPTH = 5
                state = {}  # per-qc: oaccs / osb / zall
                proj_pieces = []

                def emit_S(qc, kb):
                    lo = max(128 * (kb - 4 * qc), 0)
                    pts = []
                    for p in range(2):
                        qT = qkT_sb[p]
                        kT = qkT_sb[2 + p]
                        psS = pss_pool.tile(
                            [128, 2, 512], F32, tag="s", name=f"s{p}{qc}{kb}"
                        )
                        pt = ppool.tile(
                            [128, 2, 512], F32R, tag="pt", name=f"pt{p}{kb}"
                        )
                        pts.append(pt)
                        for hslot in range(2):
                            nc.tensor.matmul(
                                psS[:, hslot, lo:512],
                                kT[
                                    64 * hslot : 64 * hslot + 64,
                                    kb * 128 : (kb + 1) * 128,
                                ],
                                qT[
                                    64 * hslot : 64 * hslot + 64,
                                    qc * 512 + lo : (qc + 1) * 512,
                                ],
                                start=True,
                                stop=True,
                            )
                        # exp (scale=1/8 fused); diag blocks only live columns
                        nc.scalar.activation(
                            pt[:, :, lo:512],
                            psS[:, :, lo:512],
                            mybir.ActivationFunctionType.Exp,
                            scale=0.125,
                        )
                    state[(qc, kb)] = pts

                def emit_PV(qc, kb):
                    nkb = 4 * qc + 4
                    if kb == 0:
                        state[qc] = [
                            pso_pool.tile(
                                [65, 512], F32, tag="o", name=f"o{qc}_{i}"
                            )
                            for i in range(4)
                        ]
                    oaccs = state[qc]
                    off = 128 * (kb - 4 * qc)
                    pts = state.pop((qc, kb))
                    for p in range(2):
                        pt = pts[p]
                        if off >= 0:  # diagonal block: causal zeroing
                            for hslot in range(2):
                                nc.gpsimd.affine_select(
                                    pt[:, hslot, :],
                                    pt[:, hslot, :],
                                    pattern=[[1, 512]],
                                    compare_op=mybir.AluOpType.is_ge,
                                    fill=0.0,
                                    base=-off,
                                    channel_multiplier=-1,
                                )
                        lo = max(off, 0)
                        for hslot in range(2):
                            nc.tensor.matmul(
                                oaccs[2 * p + hslot][:, lo:512],
                                V1_sb[kb][:, 2 * p + hslot, :],
                                pt[:, hslot, lo:512],
                                start=(kb == 0),
                                stop=(kb == nkb - 1),
                            )
                    if kb == nkb - 1:
                        emit_normalize(qc)

                def emit_normalize(qc):
                    oaccs = state.pop(qc)
                    zall = stg.tile([128, 512], F32, tag="z", bufs=2, name=f"z{qc}")
                    osb = [
                        stg.tile(
                            [65, 512], F32, tag=f"osb{i}", bufs=2, name=f"osb{qc}_{i}"
                        )
                        for i in range(4)
                    ]
                    # Z rows first so the reciprocal starts early; then O'
                    # staging (releases the psum accumulators)
                    last = len(state) == 0
                    for i in range(4):
                        if last and i % 2 == 1:
                            nc.scalar.copy(
                                zall[32 * i : 32 * i + 1, :], oaccs[i][64:65, :]
                            )
                        else:
                            nc.vector.tensor_copy(
                                zall[32 * i : 32 * i + 1, :], oaccs[i][64:65, :]
                            )
                    for i in range(4):
                        if last and i % 2 == 1:
                            nc.scalar.copy(osb[i][0:64, :], oaccs[i][0:64, :])
                        else:
                            nc.vector.tensor_copy(osb[i][0:64, :], oaccs[i][0:64, :])
                    rall = stg.tile([128, 512], F32, tag="r", bufs=2, name=f"r{qc}")
                    rscr = stg.tile(
                        [128, 512], F32, tag="rscr", bufs=2, name=f"rscr{qc}"
                    )
                    nc.vector.reciprocal_approx_accurate(rall, zall, rscr)
                    for i in range(4):
                        p, hslot = divmod(i, 2)
                        r0 = stg.tile(
                            [1, 512], F32, tag="r0", bufs=4, name=f"r0{qc}{i}"
                        )
                        nc.vector.tensor_copy(r0, rall[32 * i : 32 * i + 1, :])
                        rb = stg.tile(
                            [64, 512], F32, tag="rb", bufs=4, name=f"rb{qc}{i}"
                        )
                        nc.gpsimd.partition_broadcast(rb, r0)
                        nc.vector.tensor_mul(
                            AT_sb[p][
                                64 * hslot : 64 * hslot + 64,
                                qc * 512 : (qc + 1) * 512,
                            ],
                            osb[i][0:64, :],
                            rb,
                        )
                    for jt2 in range(4):
                        proj_pieces.append((emit_normalize.step + 6, qc, jt2))

                def emit_proj_piece(qc, jt2):
                    psp = pss_pool.tile(
                        [128, 2, 512], F32, tag="s", name=f"pps{qc}{jt2}"
                    )
                    for sub in range(2):
                        for ft in range(2):
                            nc.tensor.matmul(
                                psp[:, sub, :],
                                wp_sb[ft][
                                    :,
                                    (2 * jt2 + sub) * 128 : (2 * jt2 + sub + 1)
                                    * 128,
                                ],
                                AT_sb[ft][:, qc * 512 : (qc + 1) * 512],
                                start=(ft == 0),
                                stop=(ft == 1),
                            )
                    ost = stg.tile(
                        [128, 2, 512], F32, tag="ost", bufs=4, name=f"ost{qc}{jt2}"
                    )
                    nc.vector.tensor_copy(ost[:, 0, :], psp[:, 0, :])
                    nc.scalar.copy(ost[:, 1, :], psp[:, 1, :])
                    ([nc.sync, nc.scalar][jt2 % 2]).dma_start(
                        out=outT[
                            jt2 * 256 : (jt2 + 1) * 256, qc * 512 : (qc + 1) * 512
                        ].rearrange("(a p) q -> p a q", a=2),
                        in_=ost,
                    )

                proj_cool = 0
                for i in range(len(steps) + DEPTH):
                    emit_normalize.step = i
                    if i < len(steps):
                        emit_S(*steps[i])
                        if (
                            proj_pieces
                            and proj_cool <= 0
                            and proj_pieces[0][0] <= i
                        ):
                            _, pqc, pjt2 = proj_pieces.pop(0)
                            emit_proj_piece(pqc, pjt2)
                            proj_cool = 2
                        else:
                            proj_cool -= 1
                    j = i - DEPTH
                    if j >= 0:
                        emit_PV(*steps[j])
                for _, pqc, pjt2 in proj_pieces:
                    emit_proj_piece(pqc, pjt2)

    nc.finalize()
    return nc


_NC_CACHE = None


def _get_nc():
    global _NC_CACHE
    if _NC_CACHE is None:
        _NC_CACHE = build_nc()
    return _NC_CACHE


def make_in_maps(x, w_qkv, w_proj):
    x = np.asarray(x, dtype=np.float32)
    w_qkv = np.asarray(w_qkv, dtype=np.float32)
    w_proj = np.asarray(w_proj, dtype=np.float32)
    ones = np.ones((128, HL), dtype=np.float32)
    in_maps = []
    for c in range(N_CORES):
        b, g = divmod(c, 4)
        cs = 256 * g
        in_maps.append(
            {
                "xT": np.ascontiguousarray(x[b].T),
                "wqk": np.ascontiguousarray(
                    np.concatenate(
                        [w_qkv[:, cs : cs + 256], w_qkv[:, D + cs : D + cs + 256]],
                        axis=1,
                    )
                ),
                "wv": np.ascontiguousarray(w_qkv[:, 2 * D + cs : 2 * D + cs + 256]),
                "wp": np.ascontiguousarray(w_proj[cs : cs + 256, :]),
                "onesc": ones,
            }
        )
    return in_maps


def assemble(results):
    out = np.empty((B, T, D), dtype=np.float32)
    for b in range(B):
        acc = results[4 * b]["outT"].astype(np.float32)
        for g in range(1, 4):
            acc = acc + results[4 * b + g]["outT"]
        out[b] = acc.T
    return out


def kernel(x, w_qkv, w_proj, trace=False):
    nc = _get_nc()
    in_maps = make_in_maps(x, w_qkv, w_proj)
    res = bass_utils.run_bass_kernel_spmd(
        nc, in_maps, core_ids=list(range(N_CORES)), trace=trace
    )
    out = assemble(res.results)
    if trace:
        kernel.last_exec_time_ns = res.exec_time_ns
        kernel.last_result = res
    return out



# revision 4
# speedup vs baseline: 1.2422x; 1.2422x over previous
"""Causal multi-head attention (B=2, T=2048, D=1024, NH=16, HD=64) on 8 trn2
NeuronCores.

Sharding: data-parallel over batch (2) x tensor-parallel over head groups (4),
Megatron-style. Core c handles batch c//4, heads 4*(c%4)..4*(c%4)+3. The host
sums the 4 partial projections per batch.

Layout is feature-on-partition throughout (x^T, qk^T, S^T [k,q], O^T, out^T).
All matmul inputs are bf16 (halves HBM traffic, enables fast weight load);
PSUM accumulation is f32.

Single software pipeline per head-pair:
  S^T matmuls (K=64, two heads row-tiled into the PE concurrently) -> exp on
  the scalar engine (scale=1/8 fused; softmax max-subtraction skipped, scores
  are O(1)) -> causal zeroing of the 128-wide diagonal window only (gpsimd
  affine_select) -> PV (two heads col-tiled, M=64 each) and Z accumulation
  (ones lhsT broadcasts Z across each head's 64 output rows, col-tiled) ->
  normalize = one DVE reciprocal + one DVE multiply per (pair, qc).
Pair 0's pipeline is fed early (only its q/k m-tiles precede it); the V
projection, pair-1 qkv m-tiles, and output projection pieces are interleaved
into the attention steps as PE filler so the tensor engine never idles long
enough for the HAM clock gate to re-throttle.
"""

import sys

if "/opt/trn_rl_repo" not in sys.path:
    sys.path.insert(0, "/opt/trn_rl_repo")

import numpy as np
import ml_dtypes
import concourse.mybir as mybir
from concourse import bacc
from concourse.tile import TileContext
from concourse import bass_utils

B, T, D = 2, 2048, 1024
NH, HD = 16, 64
N_CORES = 8

KT = D // 128  # 8 contraction tiles over model dim
TT = T // 128  # 16 t-blocks of 128

BF16 = mybir.dt.bfloat16
F32 = mybir.dt.float32
NPBF = ml_dtypes.bfloat16

DEPTH = 4  # S->PV pipeline lag in steps


def build_nc():
    nc = bacc.Bacc()
    xT = nc.dram_tensor("xT", [D, T], BF16, kind="ExternalInput")
    wqk = nc.dram_tensor("wqk", [D, 512], BF16, kind="ExternalInput")
    wv = nc.dram_tensor("wv", [D, 256], BF16, kind="ExternalInput")
    wp = nc.dram_tensor("wp", [256, D], BF16, kind="ExternalInput")
    outT = nc.dram_tensor("outT", [D, T], BF16, kind="ExternalOutput")

    with TileContext(nc) as tc:
        with (
            tc.tile_pool(name="persist", bufs=1) as pers,
            tc.tile_pool(name="stage", bufs=1) as stg,
            tc.tile_pool(name="miscp", bufs=2, space="PSUM") as misc,
        ):
            qkT = [
                pers.tile([128, T], BF16, tag=f"qkT{mt}", name=f"qkT{mt}")
                for mt in range(4)
            ]
            V_sb = [
                pers.tile([128, 256], BF16, tag=f"V{tt}", name=f"V{tt}")
                for tt in range(TT)
            ]
            AT = [
                pers.tile([128, T], BF16, tag=f"AT{p}", name=f"AT{p}")
                for p in range(2)
            ]
            wp_sb = [
                pers.tile([128, D], BF16, tag=f"wp{p}", name=f"wp{p}")
                for p in range(2)
            ]
            ones64 = pers.tile([128, 64], BF16, tag="ones", name="ones64")
            nc.vector.memset(ones64, 1.0)

            with tc.tile_pool(name="qkv_in", bufs=1) as qin:
                wqk_sb, wv_sb, xT_sb = [], [], []
                dmaq = [nc.sync, nc.scalar, nc.gpsimd]
                for kt in range(KT):
                    twqk = qin.tile([128, 512], BF16, tag=f"wqk{kt}", name=f"wqk{kt}")
                    dmaq[kt % 3].dma_start(
                        out=twqk, in_=wqk[kt * 128 : (kt + 1) * 128, :]
                    )
                    wqk_sb.append(twqk)
                    txT = qin.tile([128, T], BF16, tag=f"xT{kt}", name=f"xT{kt}")
                    xT_sb.append(txT)
                # x^T arrives in column-quarters so the first qkv matmuls can
                # start after ~1/4 of the stream
                for q in range(4):
                    for kt in range(KT):
                        dmaq[(q * KT + kt) % 3].dma_start(
                            out=xT_sb[kt][:, q * 512 : (q + 1) * 512],
                            in_=xT[
                                kt * 128 : (kt + 1) * 128,
                                q * 512 : (q + 1) * 512,
                            ],
                        )
                for kt in range(KT):
                    twv = qin.tile([128, 256], BF16, tag=f"wv{kt}", name=f"wv{kt}")
                    dmaq[kt % 3].dma_start(
                        out=twv, in_=wv[kt * 128 : (kt + 1) * 128, :]
                    )
                    wv_sb.append(twv)
                for p in range(2):
                    dmaq[p].dma_start(
                        out=wp_sb[p], in_=wp[p * 128 : (p + 1) * 128, :]
                    )

                # ---- building blocks -----------------------------------
                copy_flip = [0]

                def emit_qkq(mt, q, phase1):
                    """One [128,512] quarter of qk^T m-tile mt."""
                    psq = misc.tile([128, 512], F32, tag="mp", name=f"q{mt}_{q}")
                    for kt in range(KT):
                        nc.tensor.matmul(
                            psq,
                            wqk_sb[kt][:, mt * 128 : (mt + 1) * 128],
                            xT_sb[kt][:, q * 512 : (q + 1) * 512],
                            start=(kt == 0),
                            stop=(kt == KT - 1),
                        )
                    dst = qkT[mt][:, q * 512 : (q + 1) * 512]
                    if phase1 and copy_flip[0] % 2 == 0:
                        nc.scalar.copy(dst, psq)
                    else:
                        nc.vector.tensor_copy(dst, psq)
                    copy_flip[0] += 1

                def emit_vg(g, phase1):
                    """V for t-blocks 2g, 2g+1 (all 4 heads)."""
                    psv = misc.tile([128, 512], F32, tag="mp", name=f"v{g}")
                    for j in range(2):
                        tt = 2 * g + j
                        for kt in range(KT):
                            nc.tensor.matmul(
                                psv[:, j * 256 : (j + 1) * 256],
                                xT_sb[kt][:, tt * 128 : (tt + 1) * 128],
                                wv_sb[kt],
                                start=(kt == 0),
                                stop=(kt == KT - 1),
                            )
                    for j in range(2):
                        dst = V_sb[2 * g + j]
                        if phase1 and copy_flip[0] % 2 == 0:
                            nc.scalar.copy(dst, psv[:, j * 256 : (j + 1) * 256])
                        else:
                            nc.vector.tensor_copy(
                                dst, psv[:, j * 256 : (j + 1) * 256]
                            )
                        copy_flip[0] += 1

                def emit_proj(qc, jt2, sub):
                    c0 = (2 * jt2 + sub) * 128
                    psp = misc.tile([128, 512], F32, tag="mp", name=f"pp{qc}{jt2}{sub}")
                    for p in range(2):
                        nc.tensor.matmul(
                            psp,
                            wp_sb[p][:, c0 : c0 + 128],
                            AT[p][:, qc * 512 : (qc + 1) * 512],
                            start=(p == 0),
                            stop=(p == 1),
                        )
                    ost = stg.tile(
                        [128, 512], BF16, tag="ost", bufs=4, name=f"ost{qc}{jt2}{sub}"
                    )
                    nc.vector.tensor_copy(ost, psp)
                    nc.sync.dma_start(
                        out=outT[c0 : c0 + 128, qc * 512 : (qc + 1) * 512],
                        in_=ost,
                    )

                # ---- phase 1: pair-0 q/k m-tiles + first V blocks ------
                # consume x^T column-quarters in DMA arrival order
                for q in range(4):
                    emit_qkq(0, q, True)
                    emit_qkq(2, q, True)
                    if q < 2:
                        emit_vg(q, True)

                # ---- phases 2+3: per-pair attention pipelines ----------
                with (
                    tc.tile_pool(name="ptile", bufs=DEPTH + 2) as ppool,
                    tc.tile_pool(name="psS", bufs=2, space="PSUM") as pss,
                    tc.tile_pool(name="psO", bufs=1, space="PSUM") as pso,
                    tc.tile_pool(name="psZ", bufs=1, space="PSUM") as psz,
                ):
                    QC_ORDER = [0, 3, 2, 1]
                    steps = [
                        (qc, kb) for qc in QC_ORDER for kb in range(4 * qc + 4)
                    ]

                    def pair_phase(p, fillers):
                        pts = {}
                        cur = {}
                        proj_q = []
                        filler_q = list(fillers)

                        def emit_S(qc, kb):
                            off = 128 * (kb - 4 * qc)
                            lo = max(off, 0)
                            psS = pss.tile(
                                [128, 2, 512], F32, tag="s", name=f"s{p}{qc}{kb}"
                            )
                            qT, kT = qkT[p], qkT[2 + p]
                            for h in range(2):
                                nc.tensor.matmul(
                                    psS[:, h, lo:512],
                                    kT[
                                        64 * h : 64 * h + 64,
                                        kb * 128 : (kb + 1) * 128,
                                    ],
                                    qT[
                                        64 * h : 64 * h + 64,
                                        qc * 512 + lo : (qc + 1) * 512,
                                    ],
                                    start=True,
                                    stop=True,
                                )
                            pt = ppool.tile(
                                [128, 2, 512], BF16, tag="pt", name=f"pt{p}{qc}{kb}"
                            )
                            nc.scalar.activation(
                                pt[:, :, lo:512],
                                psS[:, :, lo:512],
                                mybir.ActivationFunctionType.Exp,
                                scale=0.125,
                            )
                            if off >= 0:
                                for h in range(2):
                                    nc.gpsimd.affine_select(
                                        pt[:, h, lo : lo + 128],
                                        pt[:, h, lo : lo + 128],
                                        pattern=[[1, 128]],
                                        compare_op=mybir.AluOpType.is_ge,
                                        fill=0.0,
                                        base=0,
                                        channel_multiplier=-1,
                                    )
                            pts[(qc, kb)] = pt

                        def emit_PV(qc, kb):
                            off = 128 * (kb - 4 * qc)
                            lo = max(off, 0)
                            if kb == 0:
                                cur["o"] = pso.tile(
                                    [128, 512], F32, tag="o", name=f"o{p}{qc}"
                                )
                                cur["z"] = psz.tile(
                                    [128, 512], F32, tag="z", name=f"z{p}{qc}"
                                )
                            oacc, zacc = cur["o"], cur["z"]
                            pt = pts.pop((qc, kb))
                            last = kb == 4 * qc + 3
                            for h in range(2):
                                nc.tensor.matmul(
                                    oacc[64 * h : 64 * h + 64, lo:512],
                                    V_sb[kb][:, (2 * p + h) * 64 : (2 * p + h + 1) * 64],
                                    pt[:, h, lo:512],
                                    start=(kb == 0),
                                    stop=last,
                                )
                            for h in range(2):
                                nc.tensor.matmul(
                                    zacc[64 * h : 64 * h + 64, lo:512],
                                    ones64,
                                    pt[:, h, lo:512],
                                    start=(kb == 0),
                                    stop=last,
                                )
                            if last:
                                zrec = stg.tile(
                                    [128, 512], F32, tag="zr", bufs=2,
                                    name=f"zr{p}{qc}",
                                )
                                nc.vector.reciprocal_approx_fast(zrec, zacc)
                                nc.vector.tensor_mul(
                                    AT[p][:, qc * 512 : (qc + 1) * 512],
                                    oacc,
                                    zrec,
                                )
                                if p == 1:
                                    for jt2 in range(4):
                                        for sub in range(2):
                                            proj_q.append((qc, jt2, sub))

                        for i in range(len(steps) + DEPTH):
                            if i < len(steps):
                                emit_S(*steps[i])
                                if proj_q:
                                    emit_proj(*proj_q.pop(0))
                                elif filler_q:
                                    filler_q.pop(0)()
                            j = i - DEPTH
                            if j >= 0:
                                emit_PV(*steps[j])
                        while proj_q:
                            emit_proj(*proj_q.pop(0))
                        while filler_q:
                            filler_q.pop(0)()

                    # pair 0: remaining V blocks, then pair-1 qk m-tiles
                    fillers0 = [
                        (lambda g=g: emit_vg(g, False)) for g in range(2, 8)
                    ] + [
                        (lambda mt=mt, q=q: emit_qkq(mt, q, False))
                        for mt in (1, 3)
                        for q in range(4)
                    ]
                    pair_phase(0, fillers0)
                    pair_phase(1, [])

    nc.finalize()
    return nc


_NC_CACHE = None


def _get_nc():
    global _NC_CACHE
    if _NC_CACHE is None:
        _NC_CACHE = build_nc()
    return _NC_CACHE


def make_in_maps(x, w_qkv, w_proj):
    x = np.asarray(x, dtype=np.float32)
    w_qkv = np.asarray(w_qkv, dtype=np.float32)
    w_proj = np.asarray(w_proj, dtype=np.float32)
    in_maps = []
    for c in range(N_CORES):
        b, g = divmod(c, 4)
        cs = 256 * g
        in_maps.append(
            {
                "xT": np.ascontiguousarray(x[b].T).astype(NPBF),
                "wqk": np.ascontiguousarray(
                    np.concatenate(
                        [w_qkv[:, cs : cs + 256], w_qkv[:, D + cs : D + cs + 256]],
                        axis=1,
                    )
                ).astype(NPBF),
                "wv": np.ascontiguousarray(
                    w_qkv[:, 2 * D + cs : 2 * D + cs + 256]
                ).astype(NPBF),
                "wp": np.ascontiguousarray(w_proj[cs : cs + 256, :]).astype(NPBF),
            }
        )
    return in_maps


def assemble(results):
    out = np.empty((B, T, D), dtype=np.float32)
    for b in range(B):
        acc = results[4 * b]["outT"].astype(np.float32)
        for g in range(1, 4):
            acc = acc + results[4 * b + g]["outT"].astype(np.float32)
        out[b] = acc.T
    return out


def kernel(x, w_qkv, w_proj, trace=False):
    nc = _get_nc()
    in_maps = make_in_maps(x, w_qkv, w_proj)
    res = bass_utils.run_bass_kernel_spmd(
        nc, in_maps, core_ids=list(range(N_CORES)), trace=trace
    )
    out = assemble(res.results)
    if trace:
        kernel.last_exec_time_ns = res.exec_time_ns
        kernel.last_result = res
    return out


# revision 6
# speedup vs baseline: 1.2733x; 1.0250x over previous
"""Causal multi-head attention (B=2, T=2048, D=1024, NH=16, HD=64) on 8 trn2
NeuronCores.

Sharding: data-parallel over batch (2) x tensor-parallel over head groups (4),
Megatron-style. Core c handles batch c//4, heads 4*(c%4)..4*(c%4)+3. The host
sums the 4 partial projections per batch.

Layout is feature-on-partition throughout (x^T, qk^T, S^T [k,q], O^T, out^T).
All matmul inputs are bf16 (halves HBM traffic, enables fast weight load);
PSUM accumulation is f32.

Single software pipeline per head-pair:
  S^T matmuls (K=64, two heads row-tiled into the PE concurrently) -> exp on
  the scalar engine (scale=1/8 fused; softmax max-subtraction skipped, scores
  are O(1)) -> causal zeroing of the 128-wide diagonal window only (gpsimd
  affine_select) -> PV (two heads col-tiled, M=64 each) and Z accumulation
  (ones lhsT broadcasts Z across each head's 64 output rows, col-tiled) ->
  normalize = one DVE reciprocal + one DVE multiply per (pair, qc).
Pair 0's pipeline is fed early (only its q/k m-tiles precede it); the V
projection, pair-1 qkv m-tiles, and output projection pieces are interleaved
into the attention steps as PE filler so the tensor engine never idles long
enough for the HAM clock gate to re-throttle.
"""

import sys

if "/opt/trn_rl_repo" not in sys.path:
    sys.path.insert(0, "/opt/trn_rl_repo")

import numpy as np
import ml_dtypes
import concourse.mybir as mybir
from concourse import bacc
from concourse.tile import TileContext
from concourse import bass_utils

B, T, D = 2, 2048, 1024
NH, HD = 16, 64
N_CORES = 8

KT = D // 128  # 8 contraction tiles over model dim
TT = T // 128  # 16 t-blocks of 128

BF16 = mybir.dt.bfloat16
F32 = mybir.dt.float32
NPBF = ml_dtypes.bfloat16

DEPTH = 4  # S->PV pipeline lag in steps


def build_nc():
    nc = bacc.Bacc()
    xT = nc.dram_tensor("xT", [D, T], BF16, kind="ExternalInput")
    wqk = nc.dram_tensor("wqk", [D, 512], BF16, kind="ExternalInput")
    wv = nc.dram_tensor("wv", [D, 256], BF16, kind="ExternalInput")
    wp = nc.dram_tensor("wp", [256, D], BF16, kind="ExternalInput")
    outT = nc.dram_tensor("outT", [D, T], BF16, kind="ExternalOutput")

    with TileContext(nc) as tc:
        with (
            tc.tile_pool(name="persist", bufs=1) as pers,
            tc.tile_pool(name="stage", bufs=1) as stg,
            tc.tile_pool(name="miscp", bufs=2, space="PSUM") as misc,
        ):
            qkT = [
                pers.tile([128, T], BF16, tag=f"qkT{mt}", name=f"qkT{mt}")
                for mt in range(4)
            ]
            V_sb = [
                pers.tile([128, 256], BF16, tag=f"V{tt}", name=f"V{tt}")
                for tt in range(TT)
            ]
            AT = [
                pers.tile([128, T], BF16, tag=f"AT{p}", name=f"AT{p}")
                for p in range(2)
            ]
            wp_sb = [
                pers.tile([128, D], BF16, tag=f"wp{p}", name=f"wp{p}")
                for p in range(2)
            ]
            ones64 = pers.tile([128, 64], BF16, tag="ones", name="ones64")
            nc.vector.memset(ones64, 1.0)

            with tc.tile_pool(name="qkv_in", bufs=1) as qin:
                wqk_sb, wv_sb, xT_sb = [], [], []
                dmaq = [nc.sync, nc.scalar, nc.gpsimd]
                # weights first (small; vg/proj depend on them), then x^T in
                # column-quarters so the first qkv matmuls start after ~1/4
                # of the stream
                for kt in range(KT):
                    twqk = qin.tile([128, 512], BF16, tag=f"wqk{kt}", name=f"wqk{kt}")
                    dmaq[kt % 3].dma_start(
                        out=twqk, in_=wqk[kt * 128 : (kt + 1) * 128, :]
                    )
                    wqk_sb.append(twqk)
                    txT = qin.tile([128, T], BF16, tag=f"xT{kt}", name=f"xT{kt}")
                    xT_sb.append(txT)
                    twv = qin.tile([128, 256], BF16, tag=f"wv{kt}", name=f"wv{kt}")
                    dmaq[(kt + 1) % 3].dma_start(
                        out=twv, in_=wv[kt * 128 : (kt + 1) * 128, :]
                    )
                    wv_sb.append(twv)
                for p in range(2):
                    dmaq[p].dma_start(
                        out=wp_sb[p], in_=wp[p * 128 : (p + 1) * 128, :]
                    )
                for q in range(4):
                    for kt in range(KT):
                        dmaq[(q * KT + kt) % 3].dma_start(
                            out=xT_sb[kt][:, q * 512 : (q + 1) * 512],
                            in_=xT[
                                kt * 128 : (kt + 1) * 128,
                                q * 512 : (q + 1) * 512,
                            ],
                        )

                # ---- building blocks -----------------------------------
                copy_flip = [0]

                def emit_qkq(mt, q, phase1):
                    """One [128,512] quarter of qk^T m-tile mt."""
                    psq = misc.tile([128, 512], F32, tag="mp", name=f"q{mt}_{q}")
                    for kt in range(KT):
                        nc.tensor.matmul(
                            psq,
                            wqk_sb[kt][:, mt * 128 : (mt + 1) * 128],
                            xT_sb[kt][:, q * 512 : (q + 1) * 512],
                            start=(kt == 0),
                            stop=(kt == KT - 1),
                        )
                    dst = qkT[mt][:, q * 512 : (q + 1) * 512]
                    if phase1 and copy_flip[0] % 2 == 0:
                        nc.scalar.copy(dst, psq)
                    else:
                        nc.vector.tensor_copy(dst, psq)
                    copy_flip[0] += 1

                def emit_vg(g, phase1):
                    """V for t-blocks 2g, 2g+1 (all 4 heads)."""
                    psv = misc.tile([128, 512], F32, tag="mp", name=f"v{g}")
                    for j in range(2):
                        tt = 2 * g + j
                        for kt in range(KT):
                            nc.tensor.matmul(
                                psv[:, j * 256 : (j + 1) * 256],
                                xT_sb[kt][:, tt * 128 : (tt + 1) * 128],
                                wv_sb[kt],
                                start=(kt == 0),
                                stop=(kt == KT - 1),
                            )
                    for j in range(2):
                        dst = V_sb[2 * g + j]
                        if phase1 and copy_flip[0] % 2 == 0:
                            nc.scalar.copy(dst, psv[:, j * 256 : (j + 1) * 256])
                        else:
                            nc.vector.tensor_copy(
                                dst, psv[:, j * 256 : (j + 1) * 256]
                            )
                        copy_flip[0] += 1

                def emit_proj(qc, jt2, sub):
                    c0 = (2 * jt2 + sub) * 128
                    psp = misc.tile([128, 512], F32, tag="mp", name=f"pp{qc}{jt2}{sub}")
                    for p in range(2):
                        nc.tensor.matmul(
                            psp,
                            wp_sb[p][:, c0 : c0 + 128],
                            AT[p][:, qc * 512 : (qc + 1) * 512],
                            start=(p == 0),
                            stop=(p == 1),
                        )
                    ost = stg.tile(
                        [128, 512], BF16, tag="ost", bufs=4, name=f"ost{qc}{jt2}{sub}"
                    )
                    nc.vector.tensor_copy(ost, psp)
                    nc.sync.dma_start(
                        out=outT[c0 : c0 + 128, qc * 512 : (qc + 1) * 512],
                        in_=ost,
                    )

                # ---- phase 1: pair-0 q/k m-tiles + first V blocks ------
                # consume x^T column-quarters in DMA arrival order
                for q in range(4):
                    emit_qkq(0, q, True)
                    emit_qkq(2, q, True)
                    if q < 2:
                        emit_vg(q, True)

                # ---- phases 2+3: per-pair attention pipelines ----------
                with (
                    tc.tile_pool(name="ptile", bufs=DEPTH + 2) as ppool,
                    tc.tile_pool(name="psS", bufs=2, space="PSUM") as pss,
                    tc.tile_pool(name="psO", bufs=1, space="PSUM") as pso,
                    tc.tile_pool(name="psZ", bufs=1, space="PSUM") as psz,
                ):
                    QC_ORDER = [0, 3, 2, 1]
                    steps = [
                        (qc, kb) for qc in QC_ORDER for kb in range(4 * qc + 4)
                    ]

                    def pair_phase(p, fillers):
                        pts = {}
                        cur = {}
                        proj_q = []
                        filler_q = list(fillers)

                        def emit_S(qc, kb):
                            off = 128 * (kb - 4 * qc)
                            lo = max(off, 0)
                            psS = pss.tile(
                                [128, 2, 512], F32, tag="s", name=f"s{p}{qc}{kb}"
                            )
                            qT, kT = qkT[p], qkT[2 + p]
                            for h in range(2):
                                nc.tensor.matmul(
                                    psS[:, h, lo:512],
                                    kT[
                                        64 * h : 64 * h + 64,
                                        kb * 128 : (kb + 1) * 128,
                                    ],
                                    qT[
                                        64 * h : 64 * h + 64,
                                        qc * 512 + lo : (qc + 1) * 512,
                                    ],
                                    start=True,
                                    stop=True,
                                )
                            pt = ppool.tile(
                                [128, 2, 512], BF16, tag="pt", name=f"pt{p}{qc}{kb}"
                            )
                            nc.scalar.activation(
                                pt[:, :, lo:512],
                                psS[:, :, lo:512],
                                mybir.ActivationFunctionType.Exp,
                                scale=0.125,
                            )
                            if off >= 0:
                                for h in range(2):
                                    nc.gpsimd.affine_select(
                                        pt[:, h, lo : lo + 128],
                                        pt[:, h, lo : lo + 128],
                                        pattern=[[1, 128]],
                                        compare_op=mybir.AluOpType.is_ge,
                                        fill=0.0,
                                        base=0,
                                        channel_multiplier=-1,
                                    )
                            pts[(qc, kb)] = pt

                        def emit_PV(qc, kb):
                            off = 128 * (kb - 4 * qc)
                            lo = max(off, 0)
                            if kb == 0:
                                cur["o"] = pso.tile(
                                    [128, 512], F32, tag="o", name=f"o{p}{qc}"
                                )
                                cur["z"] = psz.tile(
                                    [128, 512], F32, tag="z", name=f"z{p}{qc}"
                                )
                            oacc, zacc = cur["o"], cur["z"]
                            pt = pts.pop((qc, kb))
                            last = kb == 4 * qc + 3
                            for h in range(2):
                                nc.tensor.matmul(
                                    oacc[64 * h : 64 * h + 64, lo:512],
                                    V_sb[kb][:, (2 * p + h) * 64 : (2 * p + h + 1) * 64],
                                    pt[:, h, lo:512],
                                    start=(kb == 0),
                                    stop=last,
                                )
                            for h in range(2):
                                nc.tensor.matmul(
                                    zacc[64 * h : 64 * h + 64, lo:512],
                                    ones64,
                                    pt[:, h, lo:512],
                                    start=(kb == 0),
                                    stop=last,
                                )
                            if last:
                                zrec = stg.tile(
                                    [128, 512], F32, tag="zr", bufs=2,
                                    name=f"zr{p}{qc}",
                                )
                                nc.vector.reciprocal_approx_fast(zrec, zacc)
                                nc.vector.tensor_mul(
                                    AT[p][:, qc * 512 : (qc + 1) * 512],
                                    oacc,
                                    zrec,
                                )
                                if p == 1:
                                    for jt2 in range(4):
                                        for sub in range(2):
                                            proj_q.append((qc, jt2, sub))

                        for i in range(len(steps) + DEPTH):
                            if i < len(steps):
                                emit_S(*steps[i])
                                if proj_q:
                                    emit_proj(*proj_q.pop(0))
                                elif filler_q and i % 2 == 0:
                                    filler_q.pop(0)()
                            else:
                                # flush region: drain pending proj pieces
                                # between the remaining PV steps
                                for _ in range(2):
                                    if proj_q:
                                        emit_proj(*proj_q.pop(0))
                            j = i - DEPTH
                            if j >= 0:
                                emit_PV(*steps[j])
                        while proj_q:
                            emit_proj(*proj_q.pop(0))
                        while filler_q:
                            filler_q.pop(0)()

                    # pair 0: remaining V blocks, then pair-1 qk m-tiles
                    fillers0 = [
                        (lambda g=g: emit_vg(g, False)) for g in range(2, 8)
                    ] + [
                        (lambda mt=mt, q=q: emit_qkq(mt, q, False))
                        for mt in (1, 3)
                        for q in range(4)
                    ]
                    pair_phase(0, fillers0)
                    pair_phase(1, [])

    nc.finalize()
    return nc


_NC_CACHE = None


def _get_nc():
    global _NC_CACHE
    if _NC_CACHE is None:
        _NC_CACHE = build_nc()
    return _NC_CACHE


def make_in_maps(x, w_qkv, w_proj):
    x = np.asarray(x, dtype=np.float32)
    w_qkv = np.asarray(w_qkv, dtype=np.float32)
    w_proj = np.asarray(w_proj, dtype=np.float32)
    in_maps = []
    for c in range(N_CORES):
        b, g = divmod(c, 4)
        cs = 256 * g
        in_maps.append(
            {
                "xT": np.ascontiguousarray(x[b].T).astype(NPBF),
                "wqk": np.ascontiguousarray(
                    np.concatenate(
                        [w_qkv[:, cs : cs + 256], w_qkv[:, D + cs : D + cs + 256]],
                        axis=1,
                    )
                ).astype(NPBF),
                "wv": np.ascontiguousarray(
                    w_qkv[:, 2 * D + cs : 2 * D + cs + 256]
                ).astype(NPBF),
                "wp": np.ascontiguousarray(w_proj[cs : cs + 256, :]).astype(NPBF),
            }
        )
    return in_maps


def assemble(results):
    out = np.empty((B, T, D), dtype=np.float32)
    for b in range(B):
        acc = results[4 * b]["outT"].astype(np.float32)
        for g in range(1, 4):
            acc = acc + results[4 * b + g]["outT"].astype(np.float32)
        out[b] = acc.T
    return out


def kernel(x, w_qkv, w_proj, trace=False):
    nc = _get_nc()
    in_maps = make_in_maps(x, w_qkv, w_proj)
    res = bass_utils.run_bass_kernel_spmd(
        nc, in_maps, core_ids=list(range(N_CORES)), trace=trace
    )
    out = assemble(res.results)
    if trace:
        kernel.last_exec_time_ns = res.exec_time_ns
        kernel.last_result = res
    return out
